# revision 9
# baseline (speedup 1.0000x reference)
"""CTRNN forward kernel for Trainium2 (8 NeuronCores, batch-sharded).

Model (per step t):
    pre = x_t @ w_in^T + b_in + h @ w_hh^T + b_hh + sigma * n_t
    h'  = (1-a)*h + a*relu(pre)

For w_hh = d*I (uniform diagonal, the reset_parameters init) the recurrence is
elementwise:
    h' = max(ca*h + v, cc*h)    with v = a*(x w^T + b + sigma n),
                                     ca = (1-a)+a*d, cc = (1-a)

Layout trick (this is the whole kernel): per core, batch BL=16 and H=512
split into CG=4 channel groups of 128.  The recurrence runs in layout
    [c' (128 partitions), (cg, t, b)]
which the PE produces DIRECTLY: stationary lhsT = w^T chunk [i',c'],
moving rhs = x chunk [i', (t,b)] -> psum[c', (t,b)] per (cg, bank).
So there is no cross-partition corner turn anywhere:
  1. one DMA per 32-step sweep loads x [i', (kc,t,b)] (1KB descriptors)
  2. 16 PE matmuls (4 kc x 4 cg, N=512) accumulate v into 4 psum banks
  3. Pool evacuates psum + adds pre-scaled noise (loaded in the recurrence
     layout, 4KB descriptors) -> v tile (bf16)
  4. DVE: 32 fused recurrence steps, [128 x (cg,b)=64] each, f32 state
  5. ACT downcasts the f32 state tile to bf16
  6. one DMA stores the sweep's hidden states (4KB descriptors)
All layout work (transposes, scale folding) happens on the host in numpy.
"""

import os
import sys

import numpy as np

for _p in ("/opt/trn_rl_repo", os.path.expanduser("~/.axon_site/_ro/trn_rl_repo")):
    if os.path.isdir(_p) and _p not in sys.path:
        sys.path.insert(0, _p)

S, B, I, H = 1024, 128, 512, 512
TAU, DT = 100.0, 20.0
ALPHA = DT / TAU  # 0.2
SIGMA_REC = 0.05
SIGMA = float(np.sqrt(2.0 / ALPHA) * SIGMA_REC)

NCORES = 8
BL = B // NCORES  # 16 batch rows per core
CG = 4  # channel groups of 128 (H = CG*128)
KC = 4  # contraction chunks of 128 (I = KC*128)
TSW = 32  # steps per sweep (psum capacity: 4 banks x 512 f32)
NSW = S // TSW  # 32 sweeps
FS = TSW * BL  # 512 = moving free size per (kc| cg)
GB = CG * BL  # 64 = recurrence elements per partition per step

# dtype knobs
X_DT = os.environ.get("CTRNN_X_DT", "bfloat16")  # x / w matmul dtype
V_DT = os.environ.get("CTRNN_V_DT", "bfloat16")  # v (= psum + noise) dtype
O_DT = os.environ.get("CTRNN_O_DT", "bfloat16")  # output store dtype

_PROGRAM_CACHE: dict = {}
_CTRNN_OP = None


def _get_ctrnn_dve_op():
    """Register a custom fused DVE op: out = max(in0*s0 + in1, in0*s1)."""
    global _CTRNN_OP
    if _CTRNN_OP is not None:
        return _CTRNN_OP
    import concourse.dve_ops as dve_ops
    from concourse.dve_spec import C0, C1, Spec, Src0, Src1, _has_src1, lower, maxx
    from concourse.dve_uop import DveOpSpec

    name = "CTRNN_STEP_ANT"
    for existing in dve_ops.OPS:
        if existing.name == name:
            _CTRNN_OP = existing
            return existing
    spec = Spec(
        body=maxx(Src0 * C0 + Src1, Src0 * C1),
        reference=lambda in0, in1, s0, s1, imm2: np.maximum(
            in0.astype(np.float32) * s0
            + np.asarray(in1).reshape(np.shape(in0)).astype(np.float32),
            in0 * s1,
        ).astype(np.float32),
    )
    row = max(dve_ops._SUB_OPCODE_FOR_NAME.values()) + 1
    assert row < 0x20
    dve_ops._SUB_OPCODE_FOR_NAME[name] = row
    shas = {}
    for ver in ("v3", "v4"):
        try:
            shas[ver] = DveOpSpec(
                name=name, opcode=row, uops=lower(spec, ver=ver),
                rd1_en=_has_src1(spec),
            ).sha(ver)
        except Exception:
            pass
    op = dve_ops.DveOp(name, spec, subdim=False, uops_sha=shas)
    dve_ops.OPS.append(op)
    dve_ops.CUSTOM_DVE_SPECS[name] = spec
    _CTRNN_OP = op
    return op


def _build_program(n_sw: int, coef_a: float, coef_c: float, x_dt_name: str,
                   v_dt_name: str, o_dt_name: str = "bfloat16"):
    import concourse.bacc as bacc
    import concourse.mybir as mybir
    from concourse import tile

    f32 = mybir.dt.float32
    x_dt = getattr(mybir.dt, x_dt_name)
    v_dt = getattr(mybir.dt, v_dt_name)
    o_dt = getattr(mybir.dt, o_dt_name)

    nc = bacc.Bacc(
        "TRN2",
        target_bir_lowering=False,
        debug=False,
        num_devices=NCORES,
    )

    # x: [kc, i', sweep, (t,b)] — per (kc,i',sweep) a 512-elem contiguous run
    x_d = nc.dram_tensor("x_l", [KC, 128, n_sw, FS], x_dt, kind="ExternalInput")
    # noise (pre-scaled, bias-folded) in psum order: [c', sweep, (cg,t,b)]
    # (enters psum through a PE identity matmul — GPSIMD can't read PSUM)
    n_d = nc.dram_tensor("n_l", [128, n_sw, CG * FS], x_dt, kind="ExternalInput")
    # weights: [i', (kc, cg, c')] = alpha * w_in[cg*128+c', kc*128+i'],
    # plus a trailing 128x128 identity block (noise-injection stationary)
    w_d = nc.dram_tensor(
        "w_l", [128, (KC * CG + 1) * 128], x_dt, kind="ExternalInput"
    )
    # h0: [c', (cg, b)]
    h0_d = nc.dram_tensor("h0l", [128, GB], f32, kind="ExternalInput")
    # out: [c', sweep, (t, cg, b)]
    o_d = nc.dram_tensor("out_l", [128, n_sw, TSW * GB], o_dt, kind="ExternalOutput")

    mult = mybir.AluOpType.mult
    add = mybir.AluOpType.add
    copy_fn = mybir.ActivationFunctionType.Copy
    step_op = _get_ctrnn_dve_op()

    with tile.TileContext(nc) as tc:
        with (
            tc.tile_pool(name="const", bufs=1) as cpool,
            tc.tile_pool(name="xp", bufs=3) as xpool,
            tc.tile_pool(name="np", bufs=3) as npool,
            tc.tile_pool(name="ps", bufs=2, space="PSUM") as ppool,
            tc.tile_pool(name="vp", bufs=3) as vpool,
            tc.tile_pool(name="op", bufs=2) as opool,
            tc.tile_pool(name="ob", bufs=3) as obpool,
        ):
            w_sb = cpool.tile([128, (KC * CG + 1) * 128], x_dt)
            nc.sync.dma_start(out=w_sb[:], in_=w_d.ap())
            ident = w_sb[:, KC * CG * 128 :]
            h0_sb = cpool.tile([128, GB], f32)
            nc.sync.dma_start(out=h0_sb[:], in_=h0_d.ap())

            # [128, cg, b] view of the initial state
            prev = h0_sb[:].rearrange("p (cg b) -> p cg b", cg=CG)
            for s in range(n_sw):
                x_t = xpool.tile([128, KC * FS], x_dt)
                nc.sync.dma_start(
                    out=x_t[:].rearrange("p (kc f) -> p kc f", kc=KC),
                    in_=x_d.ap()[:, :, s, :].rearrange("kc p f -> p kc f"),
                )
                n_t = npool.tile([128, CG * FS], x_dt)
                nc.scalar.dma_start(out=n_t[:], in_=n_d.ap()[:, s])

                ps_t = ppool.tile([128, CG * FS], f32)
                # noise first: one shared identity stationary seeds each bank
                for cg in range(CG):
                    nc.tensor.matmul(
                        out=ps_t[:, cg * FS : (cg + 1) * FS],
                        lhsT=ident,
                        rhs=n_t[:, cg * FS : (cg + 1) * FS],
                        start=True,
                        stop=False,
                    )
                for kc in range(KC):
                    for cg in range(CG):
                        nc.tensor.matmul(
                            out=ps_t[:, cg * FS : (cg + 1) * FS],
                            lhsT=w_sb[
                                :, (kc * CG + cg) * 128 : (kc * CG + cg + 1) * 128
                            ],
                            rhs=x_t[:, kc * FS : (kc + 1) * FS],
                            start=False,
                            stop=(kc == KC - 1),
                        )

                # evacuate psum -> v (ACT engine; frees psum)
                v_t = vpool.tile([128, CG * FS], v_dt)
                nc.scalar.activation(out=v_t[:], in_=ps_t[:], func=copy_fn)

                # recurrence: 32 steps, [128 x (cg,b)] each, f32 state
                o_t = opool.tile([128, TSW * GB], f32)
                o_v = o_t[:].rearrange("p (t cg b) -> p t cg b", t=TSW, cg=CG)
                v_v = v_t[:].rearrange("p (cg t b) -> p cg t b", cg=CG, t=TSW)
                for t in range(TSW):
                    osl = o_v[:, t]  # [128, cg, b] contiguous
                    vsl = v_v[:, :, t]  # [128, cg, b] strided (cg stride FS)
                    nc.vector._custom_dve(
                        step_op, out=osl, in0=prev, in1=vsl,
                        s0=coef_a, s1=coef_c,
                    )
                    prev = osl

                # downcast (Pool) + store the sweep's hidden states
                if o_dt == f32:
                    nc.sync.dma_start(out=o_d.ap()[:, s], in_=o_t[:])
                else:
                    ob_t = obpool.tile([128, TSW * GB], o_dt)
                    nc.gpsimd.tensor_copy(out=ob_t[:], in_=o_t[:])
                    nc.sync.dma_start(out=o_d.ap()[:, s], in_=ob_t[:])

    nc.finalize()
    return nc


def _get_program(n_sw, coef_a, coef_c, x_dt_name, v_dt_name, o_dt_name="bfloat16"):
    key = (n_sw, coef_a, coef_c, x_dt_name, v_dt_name, o_dt_name)
    if key not in _PROGRAM_CACHE:
        _PROGRAM_CACHE[key] = _build_program(
            n_sw, coef_a, coef_c, x_dt_name, v_dt_name, o_dt_name
        )
    return _PROGRAM_CACHE[key]


def _np_dt(name):
    if name in ("float32", "float32r"):
        return np.float32
    import ml_dtypes

    return np.dtype(ml_dtypes.bfloat16)


def _pack_x(x_c, x_np_dt, n_sw):
    """x_c (S', BL, I) -> [kc, i', sweep, (t,b)]"""
    s_tot = n_sw * TSW
    arr = x_c.reshape(n_sw, TSW, BL, KC, 128)
    arr = arr.transpose(3, 4, 0, 1, 2).reshape(KC, 128, n_sw, FS)
    return np.ascontiguousarray(arr.astype(x_np_dt))


def _pack_noise(nhat_c, v_np_dt, n_sw):
    """pre-scaled noise (S', BL, H) -> [c', sweep, (cg,t,b)]"""
    arr = nhat_c.reshape(n_sw, TSW, BL, CG, 128)
    arr = arr.transpose(4, 0, 3, 1, 2).reshape(128, n_sw, CG * FS)
    return np.ascontiguousarray(arr.astype(v_np_dt))


def _pack_w(w_in, x_np_dt):
    """alpha*w_in (H, I) -> [i', (kc, cg, c')] ++ identity block"""
    arr = (ALPHA * w_in.astype(np.float32)).reshape(CG, 128, KC, 128)
    arr = arr.transpose(3, 2, 0, 1).reshape(128, KC * CG * 128)
    arr = np.concatenate([arr, np.eye(128, dtype=np.float32)], axis=1)
    return np.ascontiguousarray(arr.astype(x_np_dt))


def _pack_h0(h0_c):
    """h0 (BL, H) -> [c', (cg, b)]"""
    arr = h0_c.astype(np.float32).reshape(BL, CG, 128)
    arr = arr.transpose(2, 1, 0).reshape(128, GB)
    return np.ascontiguousarray(arr)


def _unpack_out(o, n_sw):
    """[c', sweep, (t, cg, b)] -> (S', BL, H)"""
    o = np.asarray(o, dtype=np.float32)
    arr = o.reshape(128, n_sw, TSW, CG, BL)
    return arr.transpose(1, 2, 4, 3, 0).reshape(n_sw * TSW, BL, H)


def _host_inputs(x, noise, w_in, b_in, b_hh, h0, x_np_dt, v_np_dt=None):
    bias = (ALPHA * (b_in + b_hh)).astype(np.float32)
    w_pack = _pack_w(w_in, x_np_dt)
    in_maps = []
    for c in range(NCORES):
        bs = slice(c * BL, (c + 1) * BL)
        nhat = (ALPHA * SIGMA) * noise[:, bs, :].astype(np.float32) + bias
        in_maps.append(
            {
                "x_l": _pack_x(x[:, bs, :].astype(np.float32), x_np_dt, NSW),
                "n_l": _pack_noise(nhat, x_np_dt, NSW),
                "w_l": w_pack,
                "h0l": _pack_h0(h0[bs]),
            }
        )
    return in_maps


def _gather_output(results):
    out = np.empty((S, B, H), dtype=np.float32)
    for c in range(NCORES):
        out[:, c * BL : (c + 1) * BL, :] = _unpack_out(results[c]["out_l"], NSW)
    return out


def _numpy_fallback(x, noise, w_in, b_in, w_hh, b_hh, h0):
    h = h0.astype(np.float32).copy()
    out = np.empty((S, B, H), dtype=np.float32)
    one_minus_a = np.float32(1.0 - ALPHA)
    a = np.float32(ALPHA)
    sg = np.float32(SIGMA)
    for t in range(S):
        pre = x[t] @ w_in.T + b_in + h @ w_hh.T + b_hh + sg * noise[t]
        h = h * one_minus_a + np.maximum(pre, 0) * a
        out[t] = h
    return out


def kernel(x, noise, w_in, b_in, w_hh, b_hh, h0):
    x = np.asarray(x, dtype=np.float32)
    noise = np.asarray(noise, dtype=np.float32)
    w_in = np.asarray(w_in, dtype=np.float32)
    b_in = np.asarray(b_in, dtype=np.float32)
    w_hh = np.asarray(w_hh, dtype=np.float32)
    b_hh = np.asarray(b_hh, dtype=np.float32)
    h0 = np.asarray(h0, dtype=np.float32)

    d = np.diagonal(w_hh)
    uniform_diag = np.all(w_hh == np.diag(d)) and np.all(d == d[0])
    if not uniform_diag:
        return _numpy_fallback(x, noise, w_in, b_in, w_hh, b_hh, h0)

    dval = float(d[0])
    coef_a = (1.0 - ALPHA) + ALPHA * dval  # 0.9 for d=0.5
    coef_c = 1.0 - ALPHA  # 0.8

    from concourse.bass_utils import run_bass_kernel_spmd

    nc = _get_program(NSW, coef_a, coef_c, X_DT, V_DT, O_DT)
    in_maps = _host_inputs(
        x, noise, w_in, b_in, b_hh, h0, _np_dt(X_DT), _np_dt(V_DT)
    )
    res = run_bass_kernel_spmd(nc, in_maps, list(range(NCORES)))
    return _gather_output(res.results)


# revision 12
# speedup vs baseline: 1.7033x; 1.7033x over previous
"""CTRNN forward kernel for Trainium2 (8 NeuronCores, batch-sharded).

Model (per step t):
    pre = x_t @ w_in^T + b_in + h @ w_hh^T + b_hh + sigma * n_t
    h'  = (1-a)*h + a*relu(pre)

For w_hh = d*I (uniform diagonal, the reset_parameters init) the recurrence is
elementwise:
    h' = max(ca*h + v, cc*h)    with v = a*(x w^T + b + sigma n),
                                     ca = (1-a)+a*d, cc = (1-a)

Layout trick (this is the whole kernel): per core, batch BL=16 and H=512
split into CG=4 channel groups of 128.  The recurrence runs in layout
    [c' (128 partitions), (cg, t, b)]
which the PE produces DIRECTLY: stationary lhsT = w^T chunk [i',c'],
moving rhs = x chunk [i', (t,b)] -> psum[c', (t,b)] per (cg, bank).
So there is no cross-partition corner turn anywhere:
  1. one DMA per 32-step sweep loads x [i', (kc,t,b)] (1KB descriptors)
  2. 16 PE matmuls (4 kc x 4 cg, N=512) accumulate v into 4 psum banks
  3. Pool evacuates psum + adds pre-scaled noise (loaded in the recurrence
     layout, 4KB descriptors) -> v tile (bf16)
  4. DVE: 32 fused recurrence steps, [128 x (cg,b)=64] each, f32 state
  5. ACT downcasts the f32 state tile to bf16
  6. one DMA stores the sweep's hidden states (4KB descriptors)
All layout work (transposes, scale folding) happens on the host in numpy.
"""

import os
import sys

import numpy as np

for _p in ("/opt/trn_rl_repo", os.path.expanduser("~/.axon_site/_ro/trn_rl_repo")):
    if os.path.isdir(_p) and _p not in sys.path:
        sys.path.insert(0, _p)

S, B, I, H = 1024, 128, 512, 512
TAU, DT = 100.0, 20.0
ALPHA = DT / TAU  # 0.2
SIGMA_REC = 0.05
SIGMA = float(np.sqrt(2.0 / ALPHA) * SIGMA_REC)

NCORES = 8
BL = B // NCORES  # 16 batch rows per core
CG = 4  # channel groups of 128 (H = CG*128)
KC = 4  # contraction chunks of 128 (I = KC*128)
TSW = 32  # steps per sweep (psum capacity: 4 banks x 512 f32)
NSW = S // TSW  # 32 sweeps
FS = TSW * BL  # 512 = moving free size per (kc| cg)
GB = CG * BL  # 64 = recurrence elements per partition per step

# dtype knobs
X_DT = os.environ.get("CTRNN_X_DT", "bfloat16")  # x / w matmul dtype
V_DT = os.environ.get("CTRNN_V_DT", "bfloat16")  # v (= psum + noise) dtype
O_DT = os.environ.get("CTRNN_O_DT", "bfloat16")  # output store dtype

_PROGRAM_CACHE: dict = {}
_CTRNN_OP = None


def _get_ctrnn_dve_op():
    """Register a custom fused DVE op: out = max(in0*s0 + in1, in0*s1)."""
    global _CTRNN_OP
    if _CTRNN_OP is not None:
        return _CTRNN_OP
    import concourse.dve_ops as dve_ops
    from concourse.dve_spec import C0, C1, Spec, Src0, Src1, _has_src1, lower, maxx
    from concourse.dve_uop import DveOpSpec

    name = "CTRNN_STEP_ANT"
    for existing in dve_ops.OPS:
        if existing.name == name:
            _CTRNN_OP = existing
            return existing
    spec = Spec(
        body=maxx(Src0 * C0 + Src1, Src0 * C1),
        reference=lambda in0, in1, s0, s1, imm2: np.maximum(
            in0.astype(np.float32) * s0
            + np.asarray(in1).reshape(np.shape(in0)).astype(np.float32),
            in0 * s1,
        ).astype(np.float32),
    )
    row = max(dve_ops._SUB_OPCODE_FOR_NAME.values()) + 1
    assert row < 0x20
    dve_ops._SUB_OPCODE_FOR_NAME[name] = row
    shas = {}
    for ver in ("v3", "v4"):
        try:
            shas[ver] = DveOpSpec(
                name=name, opcode=row, uops=lower(spec, ver=ver),
                rd1_en=_has_src1(spec),
            ).sha(ver)
        except Exception:
            pass
    op = dve_ops.DveOp(name, spec, subdim=False, uops_sha=shas)
    dve_ops.OPS.append(op)
    dve_ops.CUSTOM_DVE_SPECS[name] = spec
    _CTRNN_OP = op
    return op


def _build_program(n_sw: int, coef_a: float, coef_c: float, x_dt_name: str,
                   v_dt_name: str, o_dt_name: str = "bfloat16"):
    import concourse.bacc as bacc
    import concourse.mybir as mybir
    from concourse import tile

    f32 = mybir.dt.float32
    x_dt = getattr(mybir.dt, x_dt_name)
    v_dt = getattr(mybir.dt, v_dt_name)
    o_dt = getattr(mybir.dt, o_dt_name)

    nc = bacc.Bacc(
        "TRN2",
        target_bir_lowering=False,
        debug=False,
        num_devices=NCORES,
    )

    # x: [kc, i', sweep, (t,b)] — per (kc,i',sweep) a 512-elem contiguous run
    x_d = nc.dram_tensor("x_l", [KC, 128, n_sw, FS], x_dt, kind="ExternalInput")
    # noise (pre-scaled, bias-folded) in psum order: [c', sweep, (cg,t,b)]
    # (enters psum through a PE identity matmul — GPSIMD can't read PSUM)
    n_d = nc.dram_tensor("n_l", [128, n_sw, CG * FS], x_dt, kind="ExternalInput")
    # weights: [i', (kc, cg, c')] = alpha * w_in[cg*128+c', kc*128+i'],
    # plus a trailing 128x128 identity block (noise-injection stationary)
    w_d = nc.dram_tensor(
        "w_l", [128, (KC * CG + 1) * 128], x_dt, kind="ExternalInput"
    )
    # h0: [c', (cg, b)]
    h0_d = nc.dram_tensor("h0l", [128, GB], f32, kind="ExternalInput")
    # out: [c', sweep, (t, cg, b)]
    o_d = nc.dram_tensor("out_l", [128, n_sw, TSW * GB], o_dt, kind="ExternalOutput")

    mult = mybir.AluOpType.mult
    add = mybir.AluOpType.add
    copy_fn = mybir.ActivationFunctionType.Copy
    step_op = _get_ctrnn_dve_op()

    with tile.TileContext(nc) as tc:
        with (
            tc.tile_pool(name="const", bufs=1) as cpool,
            tc.tile_pool(name="xp", bufs=3) as xpool,
            tc.tile_pool(name="np", bufs=3) as npool,
            tc.tile_pool(name="ps", bufs=2, space="PSUM") as ppool,
            tc.tile_pool(name="vp", bufs=3) as vpool,
            tc.tile_pool(name="op", bufs=2) as opool,
            tc.tile_pool(name="ob", bufs=3) as obpool,
        ):
            w_sb = cpool.tile([128, (KC * CG + 1) * 128], x_dt)
            nc.sync.dma_start(out=w_sb[:], in_=w_d.ap())
            ident = w_sb[:, KC * CG * 128 :]
            h0_sb = cpool.tile([128, GB], f32)
            nc.sync.dma_start(out=h0_sb[:], in_=h0_d.ap())

            prev = h0_sb[:]
            for s in range(n_sw):
                x_t = xpool.tile([128, KC * FS], x_dt)
                nc.sync.dma_start(
                    out=x_t[:].rearrange("p (kc f) -> p kc f", kc=KC),
                    in_=x_d.ap()[:, :, s, :].rearrange("kc p f -> p kc f"),
                )
                n_t = npool.tile([128, CG * FS], x_dt)
                nc.scalar.dma_start(out=n_t[:], in_=n_d.ap()[:, s])

                ps_t = ppool.tile([128, CG * FS], f32)
                # noise first: one shared identity stationary seeds each bank
                for cg in range(CG):
                    nc.tensor.matmul(
                        out=ps_t[:, cg * FS : (cg + 1) * FS],
                        lhsT=ident,
                        rhs=n_t[:, cg * FS : (cg + 1) * FS],
                        start=True,
                        stop=False,
                    )
                for kc in range(KC):
                    for cg in range(CG):
                        nc.tensor.matmul(
                            out=ps_t[:, cg * FS : (cg + 1) * FS],
                            lhsT=w_sb[
                                :, (kc * CG + cg) * 128 : (kc * CG + cg + 1) * 128
                            ],
                            rhs=x_t[:, kc * FS : (kc + 1) * FS],
                            start=False,
                            stop=(kc == KC - 1),
                        )

                # evacuate psum -> v in (t, cg, b) order (ACT engine; the
                # corner goes on ACT's write side so every DVE operand is
                # contiguous)
                v_t = vpool.tile([128, TSW * GB], v_dt)
                nc.scalar.activation(
                    out=v_t[:].rearrange("p (t cg b) -> p cg t b", t=TSW, cg=CG),
                    in_=ps_t[:].rearrange("p (cg t b) -> p cg t b", cg=CG, t=TSW),
                    func=copy_fn,
                )

                # recurrence: 32 steps, [128 x (cg,b)] each, f32 state
                o_t = opool.tile([128, TSW * GB], f32)
                for t in range(TSW):
                    osl = o_t[:, t * GB : (t + 1) * GB]
                    vsl = v_t[:, t * GB : (t + 1) * GB]
                    nc.vector._custom_dve(
                        step_op, out=osl, in0=prev, in1=vsl,
                        s0=coef_a, s1=coef_c,
                    )
                    prev = osl

                # downcast (ACT) + store the sweep's hidden states
                if o_dt == f32:
                    nc.sync.dma_start(out=o_d.ap()[:, s], in_=o_t[:])
                else:
                    ob_t = obpool.tile([128, TSW * GB], o_dt)
                    nc.scalar.activation(out=ob_t[:], in_=o_t[:], func=copy_fn)
                    nc.sync.dma_start(out=o_d.ap()[:, s], in_=ob_t[:])

    nc.finalize()
    return nc


def _get_program(n_sw, coef_a, coef_c, x_dt_name, v_dt_name, o_dt_name="bfloat16"):
    key = (n_sw, coef_a, coef_c, x_dt_name, v_dt_name, o_dt_name)
    if key not in _PROGRAM_CACHE:
        _PROGRAM_CACHE[key] = _build_program(
            n_sw, coef_a, coef_c, x_dt_name, v_dt_name, o_dt_name
        )
    return _PROGRAM_CACHE[key]


def _np_dt(name):
    if name in ("float32", "float32r"):
        return np.float32
    import ml_dtypes

    return np.dtype(ml_dtypes.bfloat16)


def _pack_x(x_c, x_np_dt, n_sw):
    """x_c (S', BL, I) -> [kc, i', sweep, (t,b)]"""
    s_tot = n_sw * TSW
    arr = x_c.reshape(n_sw, TSW, BL, KC, 128)
    arr = arr.transpose(3, 4, 0, 1, 2).reshape(KC, 128, n_sw, FS)
    return np.ascontiguousarray(arr.astype(x_np_dt))


def _pack_noise(nhat_c, v_np_dt, n_sw):
    """pre-scaled noise (S', BL, H) -> [c', sweep, (cg,t,b)]"""
    arr = nhat_c.reshape(n_sw, TSW, BL, CG, 128)
    arr = arr.transpose(4, 0, 3, 1, 2).reshape(128, n_sw, CG * FS)
    return np.ascontiguousarray(arr.astype(v_np_dt))


def _pack_w(w_in, x_np_dt):
    """alpha*w_in (H, I) -> [i', (kc, cg, c')] ++ identity block"""
    arr = (ALPHA * w_in.astype(np.float32)).reshape(CG, 128, KC, 128)
    arr = arr.transpose(3, 2, 0, 1).reshape(128, KC * CG * 128)
    arr = np.concatenate([arr, np.eye(128, dtype=np.float32)], axis=1)
    return np.ascontiguousarray(arr.astype(x_np_dt))


def _pack_h0(h0_c):
    """h0 (BL, H) -> [c', (cg, b)]"""
    arr = h0_c.astype(np.float32).reshape(BL, CG, 128)
    arr = arr.transpose(2, 1, 0).reshape(128, GB)
    return np.ascontiguousarray(arr)


def _unpack_out(o, n_sw):
    """[c', sweep, (t, cg, b)] -> (S', BL, H)"""
    o = np.asarray(o, dtype=np.float32)
    arr = o.reshape(128, n_sw, TSW, CG, BL)
    return arr.transpose(1, 2, 4, 3, 0).reshape(n_sw * TSW, BL, H)


def _host_inputs(x, noise, w_in, b_in, b_hh, h0, x_np_dt, v_np_dt=None):
    bias = (ALPHA * (b_in + b_hh)).astype(np.float32)
    w_pack = _pack_w(w_in, x_np_dt)
    in_maps = []
    for c in range(NCORES):
        bs = slice(c * BL, (c + 1) * BL)
        nhat = (ALPHA * SIGMA) * noise[:, bs, :].astype(np.float32) + bias
        in_maps.append(
            {
                "x_l": _pack_x(x[:, bs, :].astype(np.float32), x_np_dt, NSW),
                "n_l": _pack_noise(nhat, x_np_dt, NSW),
                "w_l": w_pack,
                "h0l": _pack_h0(h0[bs]),
            }
        )
    return in_maps


def _gather_output(results):
    out = np.empty((S, B, H), dtype=np.float32)
    for c in range(NCORES):
        out[:, c * BL : (c + 1) * BL, :] = _unpack_out(results[c]["out_l"], NSW)
    return out


def _numpy_fallback(x, noise, w_in, b_in, w_hh, b_hh, h0):
    h = h0.astype(np.float32).copy()
    out = np.empty((S, B, H), dtype=np.float32)
    one_minus_a = np.float32(1.0 - ALPHA)
    a = np.float32(ALPHA)
    sg = np.float32(SIGMA)
    for t in range(S):
        pre = x[t] @ w_in.T + b_in + h @ w_hh.T + b_hh + sg * noise[t]
        h = h * one_minus_a + np.maximum(pre, 0) * a
        out[t] = h
    return out


def kernel(x, noise, w_in, b_in, w_hh, b_hh, h0):
    x = np.asarray(x, dtype=np.float32)
    noise = np.asarray(noise, dtype=np.float32)
    w_in = np.asarray(w_in, dtype=np.float32)
    b_in = np.asarray(b_in, dtype=np.float32)
    w_hh = np.asarray(w_hh, dtype=np.float32)
    b_hh = np.asarray(b_hh, dtype=np.float32)
    h0 = np.asarray(h0, dtype=np.float32)

    d = np.diagonal(w_hh)
    uniform_diag = np.all(w_hh == np.diag(d)) and np.all(d == d[0])
    if not uniform_diag:
        return _numpy_fallback(x, noise, w_in, b_in, w_hh, b_hh, h0)

    dval = float(d[0])
    coef_a = (1.0 - ALPHA) + ALPHA * dval  # 0.9 for d=0.5
    coef_c = 1.0 - ALPHA  # 0.8

    from concourse.bass_utils import run_bass_kernel_spmd

    nc = _get_program(NSW, coef_a, coef_c, X_DT, V_DT, O_DT)
    in_maps = _host_inputs(
        x, noise, w_in, b_in, b_hh, h0, _np_dt(X_DT), _np_dt(V_DT)
    )
    res = run_bass_kernel_spmd(nc, in_maps, list(range(NCORES)))
    return _gather_output(res.results)
